# revision 26
# baseline (speedup 1.0000x reference)
"""GDFN (gated dual-branch FFN) Trainium2 kernel, 8-core SPMD.

Reference computation (per batch b):
  h = w_in @ x          (1x1 conv, 64 -> 510 ch)
  gate, x_sp, x_fr = split(h, 3)
  sp = depthwise3x3(x_sp, w_dw)                     # SAME padding
  fr = per-8x8-patch spectral op on x_fr            # irfft2(rfft2(.)*w)
  out = w_out @ (gelu_exact(gate) * (sp + fr))

Sharding: 8 cores = 4 batch x 2 H-halves (128 rows each + 1-row halo).

Per-core design (all-bf16 matmuls, N=512 wherever possible so LDWEIGHTS
hides behind streaming; PE measured to run such streams at 2.4 GHz):
  - gate / sp as bf16 matmuls streaming 2-row windows (N=512, exactly
    one PSUM bank) with K=64 row-packed x2 via tile_position; dwconv
    folded into proj (9 shifted taps, PSUM accumulation); the 42-ch
    chunk packs 2 taps in array column halves (out partitions 0-41 /
    64-105, summed at evacuation).
  - fr branch projected directly into patch-transposed layout: the
    8x16 super-patch x-window (3-level strided AP) is the stationary
    operand, w_frT streams -> out[pair-pixel, channel]. No forward
    transpose, no scatter.
  - freq op = per-channel blockdiag matmul (M_c precomputed host-side),
    then PE transposes back (T1'), DVE accumulates into sp acc.
  - gelu on ACT from PSUM; evacuations spread over ACT/DVE/Pool.
  - proj_out as bf16 matmuls (K = 128 + 42 accumulation).
"""

import numpy as np
from contextlib import ExitStack

import concourse.bacc as bacc
import concourse.bass as bass
import concourse.mybir as mybir
import concourse.tile as tile
import concourse.masks as masks
from concourse import bass_utils

dt = mybir.dt
AF = mybir.ActivationFunctionType
ALU = mybir.AluOpType

DIM = 64
HID = 170
P = 8
H = 256
W = 256
B = 4
N_CORES = 8

ROWS = 128          # interior rows per core slab
WPAD = W + 2        # column-padded width
QROWS = 32          # rows per quarter
NQ = ROWS // QROWS  # 4 quarters
HROWS = 16          # rows per pixel-half (row-packing)
PAIRS_Q = 64        # super-patch pairs per quarter (4 patch-rows x 16)

_bf16 = None


def _np_bf16():
    global _bf16
    if _bf16 is None:
        _bf16 = dt.np(dt.bfloat16)
    return _bf16


# ----------------------------------------------------------------------------
# host-side weight preparation
# ----------------------------------------------------------------------------

def _prep_weights(w_in, w_dw, fft_weight, w_out):
    bf16 = _np_bf16()
    w_gate = w_in[0:HID]            # [170, 64]
    w_sp = w_in[HID:2 * HID]        # [170, 64]
    w_fr = w_in[2 * HID:3 * HID]    # [170, 64]

    # gate lhsT, K=64 duplicated on partitions 0-63 / 64-127
    w_gT = np.ascontiguousarray(w_gate.T)                 # [64, 170]
    w_g_dup = np.concatenate([w_gT, w_gT], axis=0).astype(bf16)

    # fr rhs (moving operand for the transposed projection)
    w_frT = np.ascontiguousarray(w_fr.T)                  # [64, 170]
    w_fr_dup = np.concatenate([w_frT, w_frT], axis=0).astype(bf16)

    # dense-folded dwconv: per tap (di,dj): K_t[c,k] = w_dw[c,di,dj]*w_sp[c,k]
    w_dw3 = w_dw.reshape(HID, 3, 3)
    taps = []
    for di in (-1, 0, 1):
        for dj in (-1, 0, 1):
            kt = (w_dw3[:, di + 1, dj + 1:dj + 2] * w_sp)   # [170, 64]
            taps.append(np.ascontiguousarray(kt.T))         # [64, 170]
    w_sp9T = np.concatenate(taps, axis=1)                   # [64, 1530]
    w_sp9_dup = np.concatenate([w_sp9T, w_sp9T], axis=0).astype(bf16)

    # frequency-branch per-channel operator: MT_c[a_in, a_out] [64, 64]
    # (row-major 8x8 pixel order). Pair-pixel index p = r*16 + x with
    # x = pi*8 + xc (row-major over the 8x16 super-patch), so m2 is the
    # permuted block-diagonal over the two patches of a pair.
    E = np.eye(64, dtype=np.float64).reshape(64, P, P)
    F = np.fft.rfft2(E)                                     # [64, 8, 5]
    wc = fft_weight.reshape(HID, 1, P, P // 2 + 1).astype(np.float64)
    Y = np.fft.irfft2(F[None, :, :, :] * wc, s=(P, P))      # [170, 64, 8, 8]
    MT = Y.reshape(HID, 64, 64)                             # [c, a_in, a_out]
    idx = np.arange(128)
    r_, x_ = idx // 16, idx % 16
    pi_ = x_ // P
    f_ = r_ * P + (x_ % P)
    m2 = np.zeros((HID, 128, 128), dtype=np.float64)
    same = pi_[:, None] == pi_[None, :]
    m2[:, same] = MT[:, f_[np.where(same)[0]], f_[np.where(same)[1]]]
    m2 = np.ascontiguousarray(
        m2.transpose(1, 0, 2).reshape(128, HID * 128)).astype(bf16)

    w_outT = np.ascontiguousarray(w_out.T)                  # [170, 64]
    w_oa = w_outT[0:128].astype(bf16)
    w_ob = w_outT[128:HID].astype(bf16)
    return {
        "w_g": w_g_dup,
        "w_fr": w_fr_dup,
        "w_sp9": w_sp9_dup,
        "m2": m2,
        "w_oa": w_oa,
        "w_ob": w_ob,
    }


def _prep_slabs(x):
    """x [4, 64, 256, 256] -> 8 bf16 slabs [64, 130, 258] (zero halos)
    plus patch-major slabs [64, 256 pairs, 128] for the fr branch
    (pair g = q*64 + half*32 + pr_h*16 + cp, pixel order r*16+x)."""
    bf16 = _np_bf16()
    slabs = []
    for b in range(B):
        for hh in range(2):
            r0 = hh * ROWS
            sl = np.zeros((DIM, ROWS + 2, WPAD), dtype=bf16)
            lo = max(r0 - 1, 0)
            hi = min(r0 + ROWS + 1, H)
            sl[:, lo - (r0 - 1):hi - (r0 - 1), 1:W + 1] = x[b, :, lo:hi, :]
            interior = np.asarray(x[b, :, r0:r0 + ROWS, :], dtype=bf16)
            # [64, (q, half, pr_h, r=8), (cp, x=16)]
            xp = interior.reshape(DIM, 16, 8, 16, 16)
            xp = np.ascontiguousarray(xp.transpose(0, 1, 3, 2, 4))
            slabs.append((sl, xp.reshape(DIM, 256, 128)))
    return slabs


# ----------------------------------------------------------------------------
# device program
# ----------------------------------------------------------------------------

def _build_program():
    nc = bacc.Bacc("TRN2", target_bir_lowering=False, debug=False,
                   num_devices=N_CORES)

    xs_d = nc.dram_tensor("xs", [DIM, ROWS + 2, WPAD], dt.bfloat16,
                          kind="ExternalInput")
    xp_d = nc.dram_tensor("xp", [DIM, 256 * 128], dt.bfloat16,
                          kind="ExternalInput")
    wg_d = nc.dram_tensor("w_g", [128, HID], dt.bfloat16,
                          kind="ExternalInput")
    wfr_d = nc.dram_tensor("w_fr", [128, HID], dt.bfloat16,
                           kind="ExternalInput")
    wsp_d = nc.dram_tensor("w_sp9", [128, 9 * HID], dt.bfloat16,
                           kind="ExternalInput")
    m2_d = nc.dram_tensor("m2", [128, HID * 128], dt.bfloat16,
                          kind="ExternalInput")
    woa_d = nc.dram_tensor("w_oa", [128, 64], dt.bfloat16,
                           kind="ExternalInput")
    wob_d = nc.dram_tensor("w_ob", [42, 64], dt.bfloat16,
                           kind="ExternalInput")
    out_d = nc.dram_tensor("out", [DIM, ROWS, W], dt.float32,
                           kind="ExternalOutput")

    with tile.TileContext(nc) as tc, ExitStack() as ctx:
        const = ctx.enter_context(tc.tile_pool(name="const", bufs=1))
        xpool = ctx.enter_context(tc.tile_pool(name="xp", bufs=2))
        hpool = ctx.enter_context(tc.tile_pool(name="hp", bufs=1))
        obuf = ctx.enter_context(tc.tile_pool(name="ob", bufs=3))
        ps_mm = ctx.enter_context(tc.tile_pool(name="psmm", bufs=3,
                                               space="PSUM"))
        ps_fr = ctx.enter_context(tc.tile_pool(name="psfr", bufs=2,
                                               space="PSUM"))
        ps_t1 = ctx.enter_context(tc.tile_pool(name="pst1", bufs=3,
                                               space="PSUM"))

        # constants
        w_g = const.tile([128, HID], dt.bfloat16)
        w_fr = const.tile([128, HID], dt.bfloat16)
        w_sp9 = const.tile([128, 9 * HID], dt.bfloat16)
        m2 = const.tile([128, HID * 128], dt.bfloat16)
        w_oa = const.tile([128, 64], dt.bfloat16)
        w_ob = const.tile([42, 64], dt.bfloat16)
        ident = const.tile([128, 128], dt.bfloat16)
        nc.sync.dma_start(w_g[:], wg_d.ap())
        nc.sync.dma_start(w_fr[:], wfr_d.ap())
        nc.sync.dma_start(w_sp9[:], wsp_d.ap())
        nc.sync.dma_start(m2[:], m2_d.ap())
        nc.sync.dma_start(w_oa[:], woa_d.ap())
        nc.sync.dma_start(w_ob[:], wob_d.ap())
        masks.make_identity(nc, ident[:])

        # per-quarter persistent tiles
        g1 = hpool.tile([128, QROWS * W], dt.bfloat16, tag="g1")
        g2 = hpool.tile([42, QROWS * W], dt.bfloat16, tag="g2")
        acc1 = hpool.tile([128, QROWS * W], dt.bfloat16, tag="a1")
        acc2 = hpool.tile([42, QROWS * W], dt.bfloat16, tag="a2")
        Bt = hpool.tile([128, HID * PAIRS_Q], dt.bfloat16, tag="B")
        Btf = hpool.tile([128, HID * PAIRS_Q], dt.bfloat16, tag="Bf")

        xs = xs_d.ap().rearrange("c r w -> c (r w)")

        def load_xp(qq):
            xq = xpool.tile([128, 32 * 128], dt.bfloat16, tag="xq",
                            name=f"xp_{qq}")
            nc.sync.dma_start(
                xq[0:64, :],
                xp_d.ap()[:, (qq * 64) * 128:(qq * 64 + 32) * 128])
            nc.sync.dma_start(
                xq[64:128, :],
                xp_d.ap()[:, (qq * 64 + 32) * 128:(qq * 64 + 64) * 128])
            return xq

        def pslc(h):
            return (0, 64) if h == 0 else (64, 128)

        Brd = Bt[:].rearrange("p (j c) -> p c j", c=HID)
        Bfw = Btf[:].rearrange("p (j c) -> p c j", c=HID)

        def emit_A(q, xq):
            """fr transposed projection -> Bt. Super-patch pair =
            8x16 px; stationary operand = contiguous patch-major x
            window, pair-pixel order p = r*16 + x. PSUM groups of 3
            pairs (510 fp32 = one bank), one live group per half;
            halves alternate row-groups so LDWEIGHTS overlaps the
            other half's matmul."""
            pf_cur = [None, None]
            for pp in range(32):
                for half in range(2):
                    j = half * 32 + pp
                    sl = pp % 3
                    ngrp = 3 if pp < 30 else 2
                    p0, p1 = pslc(half)
                    if sl == 0:
                        pf_cur[half] = ps_fr.tile(
                            [128, 510], dt.float32, tag="fr",
                            name=f"pfr_{q}_{half}_{pp}")
                    nc.tensor.matmul(
                        pf_cur[half][:, sl * 170:sl * 170 + 170],
                        xq[p0:p1, pp * 128:pp * 128 + 128],
                        w_fr[p0:p1, :], start=True, stop=True)
                    if sl == ngrp - 1:
                        j0 = j - sl
                        ev = pf_cur[half][:, 0:ngrp * 170]
                        dst = Bt[:, j0 * 170:(j0 + ngrp) * 170]
                        if (half + pp // 3) % 2 == 0:
                            nc.scalar.activation(dst, ev, AF.Copy)
                        else:
                            nc.vector.tensor_copy(dst, ev)

        def emit_B(q):
            """gate + sp stripes with full-K freq matmuls interleaved.
            The two 16-row halves run as concurrent PE streams on row
            groups (0,0)/(64,0); matmuls are half-interleaved AND
            M=128/M=42 chunks alternate so PE array activity never
            dips long enough for HAM to re-throttle."""
            xt = xpool.tile([128, 18 * WPAD], dt.bfloat16, tag="xt",
                            name=f"xt_{q}")
            r_a = q * QROWS
            r_b = q * QROWS + HROWS
            nc.sync.dma_start(
                xt[0:64, :], xs[:, r_a * WPAD:(r_a + 18) * WPAD])
            nc.sync.dma_start(
                xt[64:128, :], xs[:, r_b * WPAD:(r_b + 18) * WPAD])
            xv = xt[:].rearrange("p (r w) -> p r w", w=WPAD)

            def rhs2(half, rr, di=0, dj=0):
                p0, p1 = pslc(half)
                return xv[p0:p1, rr + 1 + di:rr + 3 + di,
                          1 + dj:W + 1 + dj]

            fgroups = list(range(0, HID, 8))
            fplan = [3, 3, 3, 3, 3, 3, 2, 2]
            fg_idx = 0
            for rb in range(8):
                rr = rb * 2
                pg1, pg2, psp1, psp2 = {}, {}, {}, {}
                for half in range(2):
                    p0, p1 = pslc(half)
                    pg1[half] = ps_t1.tile([128, 512], dt.float32,
                                           tag="t1",
                                           name=f"pg1_{q}_{rb}_{half}")
                    nc.tensor.matmul(pg1[half][:], w_g[p0:p1, 0:128],
                                     rhs2(half, rr), start=True,
                                     stop=True)
                for half in range(2):
                    p0, p1 = pslc(half)
                    pg2[half] = ps_fr.tile([42, 512], dt.float32,
                                           tag="fr",
                                           name=f"pg2_{q}_{rb}_{half}")
                    nc.tensor.matmul(pg2[half][:], w_g[p0:p1, 128:170],
                                     rhs2(half, rr), start=True,
                                     stop=True)
                for half in range(2):
                    psp1[half] = ps_mm.tile([128, 512], dt.float32,
                                            tag="mm",
                                            name=f"psp1_{q}_{rb}_{half}")
                    psp2[half] = ps_mm.tile([42, 512], dt.float32,
                                            tag="mm",
                                            name=f"psp2_{q}_{rb}_{half}")
                for t in range(9):
                    di, dj = t // 3 - 1, t % 3 - 1
                    for half in range(2):
                        p0, p1 = pslc(half)
                        nc.tensor.matmul(
                            psp1[half][:],
                            w_sp9[p0:p1, t * HID:t * HID + 128],
                            rhs2(half, rr, di, dj),
                            start=(t == 0), stop=(t == 8))
                    for half in range(2):
                        p0, p1 = pslc(half)
                        nc.tensor.matmul(
                            psp2[half][:],
                            w_sp9[p0:p1, t * HID + 128:t * HID + 170],
                            rhs2(half, rr, di, dj),
                            start=(t == 0), stop=(t == 8))

                for half in range(2):
                    px0 = half * (HROWS * W) + rr * W
                    nc.scalar.activation(
                        g1[:, px0:px0 + 512], pg1[half][:], AF.Gelu)
                    nc.scalar.activation(
                        g2[:, px0:px0 + 512], pg2[half][:], AF.Gelu)
                    nc.vector.tensor_copy(
                        acc1[:, px0:px0 + 512], psp1[half][:])
                    nc.vector.tensor_copy(
                        acc2[:, px0:px0 + 512], psp2[half][:])

                for _ in range(fplan[rb]):
                    c8 = fgroups[fg_idx]
                    fg_idx += 1
                    nch = min(8, HID - c8)
                    pq = ps_t1.tile([128, 512], dt.float32, tag="t1",
                                    name=f"pq_{q}_{c8}")
                    for ci in range(nch):
                        c = c8 + ci
                        nc.tensor.matmul(
                            pq[:, ci * 64:ci * 64 + 64],
                            m2[:, c * 128:c * 128 + 128],
                            Brd[:, c, :], start=True, stop=True)
                    dst = Bfw[:, c8:c8 + nch, :]
                    src = pq[:, 0:nch * 64].rearrange(
                        "p (c j) -> p c j", c=nch)
                    if (c8 // 8) % 2 == 0:
                        nc.scalar.activation(dst, src, AF.Copy)
                    else:
                        nc.vector.tensor_copy(dst, src)

        def emit_CD(q):
            """T1' + accumulate + per-block gating, proj_out woven in
            so skinny M=64 po matmuls sit between dense transposes."""
            for pr2 in range(4):
                for cp0 in range(0, 16, 4):
                    jg = pr2 * 16 + cp0
                    pa = ps_fr.tile([128, 512], dt.bfloat16, tag="fr",
                                    name=f"pa_{q}_{jg}")
                    pb = ps_t1.tile([42, 512], dt.bfloat16, tag="t1",
                                    name=f"pb_{q}_{jg}")
                    pav = pa[:].rearrange("p (r x) -> p r x", x=64)
                    pbv = pb[:].rearrange("p (r x) -> p r x", x=64)
                    for ji in range(4):
                        j = jg + ji
                        nc.tensor.transpose(
                            pav[:, :, ji * 16:ji * 16 + 16],
                            Btf[:, j * 170:j * 170 + 128], ident[:])
                        nc.tensor.transpose(
                            pbv[:, :, ji * 16:ji * 16 + 16],
                            Btf[:, j * 170 + 128:j * 170 + 170],
                            ident[:])
                    for dst_t, src in ((acc1, pav), (acc2, pbv)):
                        d = dst_t[:].rearrange(
                            "p (rg w) -> p rg w", w=W)[
                            :, pr2 * 8:pr2 * 8 + 8,
                            cp0 * 16:cp0 * 16 + 64]
                        nc.vector.scalar_tensor_tensor(
                            out=d, in0=src, scalar=1.0,
                            in1=d, op0=ALU.mult, op1=ALU.add)
                blk = slice(pr2 * 2048, pr2 * 2048 + 2048)
                nc.gpsimd.tensor_mul(acc1[:, blk], acc1[:, blk],
                                     g1[:, blk])
                nc.gpsimd.tensor_mul(acc2[:, blk], acc2[:, blk],
                                     g2[:, blk])
                for nk in range(pr2 * 4, pr2 * 4 + 4):
                    po = ps_mm.tile([64, 512], dt.float32, tag="mm",
                                    name=f"po_{q}_{nk}")
                    nc.tensor.matmul(po[:], w_oa[:],
                                     acc1[:, nk * 512:nk * 512 + 512],
                                     start=True, stop=False)
                    nc.tensor.matmul(po[:], w_ob[:],
                                     acc2[:, nk * 512:nk * 512 + 512],
                                     start=False, stop=True)
                    ot = obuf.tile([64, 512], dt.float32, tag="o")
                    if nk % 2 == 0:
                        nc.scalar.activation(ot[:], po[:], AF.Copy)
                    else:
                        nc.vector.tensor_copy(ot[:], po[:])
                    r0 = q * QROWS + 2 * nk
                    nc.sync.dma_start(
                        out_d.ap().rearrange("c r w -> c (r w)")[
                            :, r0 * W:(r0 + 2) * W], ot[:])

        xq_cur = load_xp(0)
        emit_A(0, xq_cur)
        for q in range(NQ):
            emit_B(q)
            if q < NQ - 1:
                xq_cur = load_xp(q + 1)
                emit_A(q + 1, xq_cur)
            emit_CD(q)

    nc.compile()
    return nc


_PROGRAM = None


def _get_program():
    global _PROGRAM
    if _PROGRAM is None:
        _PROGRAM = _build_program()
    return _PROGRAM


def kernel(x, w_in, w_dw, fft_weight, w_out, _trace=False):
    x = np.asarray(x, dtype=np.float32)
    w_in = np.asarray(w_in, dtype=np.float32)
    w_dw = np.asarray(w_dw, dtype=np.float32)
    fft_weight = np.asarray(fft_weight, dtype=np.float32)
    w_out = np.asarray(w_out, dtype=np.float32)

    nc = _get_program()
    wts = _prep_weights(w_in, w_dw, fft_weight, w_out)
    slabs = _prep_slabs(x)
    in_maps = [dict(xs=slabs[i][0], xp=slabs[i][1].reshape(DIM, -1), **wts)
               for i in range(N_CORES)]
    res = bass_utils.run_bass_kernel_spmd(
        nc, in_maps, core_ids=list(range(N_CORES)), trace=_trace)

    out = np.empty((B, DIM, H, W), dtype=np.float32)
    for i in range(N_CORES):
        b, hh = i // 2, i % 2
        out[b, :, hh * ROWS:(hh + 1) * ROWS, :] = res.results[i]["out"]
    if _trace:
        kernel.last_exec_time_ns = res.exec_time_ns
    return out
